# Initial kernel scaffold
#
"""
Trainium2 Bass kernel for nn_EventMotionModel (dense transformer block).

Math (per token, B*T=65536 tokens total, hidden H=1024):
    x   = concat(state, cond)            # clip(+-16) is a provable no-op for randn inputs
    h1  = relu(LN(x @ ew1 + eb1))
    h   = relu(LN(h1 @ ew2 + eb2))
    res = x @ rw + rb
    fh  = gelu(cond @ fw1 + fb1); g,b = split(fh @ fw2 + fb2)
    qin = LN_q(h) * (1+0.5*tanh(g)) + 0.5*tanh(b)
    q/k/v = qin@wq, h@wk, h@wv ; per-head (8 heads, dh=128) attention over T=32
    h2  = LN(h + attn_out@wo + res)
    out = relu(h2 @ hw1 + hb1) @ hw2 + hb2

Kernel strategy:
  * Pure batch data parallel over 8 NeuronCores (B=2048 -> 256 per core).
  * Feature-major activations in SBUF: [feature -> partitions (8 chunks of
    128), tokens -> free dim]. Every dense layer is then
        matmul(psum[m,tok], lhsT=W[kchunk, mchunk], rhs=act[kchunk, tok])
    with the weight in its natural [in,out] layout and NO transposes anywhere.
    The host pre-transposes x once; the V projection and the final matmul use
    the activation as the stationary operand, which emits token-major output
    directly (so V is ready for attention and the result DMAs out contiguous).
  * Matmul operands are bf16 (full 78.6 TF/s PE rate; plain fp32 matmul is
    4x slower and its fused 4-byte weight-load path cannot carry the multiple
    sync waits Tile emits). PSUM accumulation and the whole LayerNorm /
    softmax normalization chain stay fp32.
  * LayerNorm mean/var across the partition (feature) dim via an all-ones
    stationary matmul (accumulated over the 8 feature chunks) -> per-token
    sums broadcast across all 128 partitions; x^2 via ACT Square.
  * Attention (T=32 per batch item): scores computed transposed
    [k-tokens, q-tokens] straight from feature-major k (stationary) and q
    (moving). Softmax over the partition dim: exp on ACT (no max subtraction:
    |scores| < ~2 for these weight scales), denominators via a block-diagonal
    ones matmul, then masked reciprocal multiply. o = attn @ v uses
    token-major v as stationary, emitting feature-major o chunks.
  * 16 token tiles of 512 per core via tc.For_i; all weights (16.5MB bf16)
    stream from HBM every tile through a triple-buffered pool, hidden under
    PE work. A BIR-JSON post-pass splits multi-sync-wait instructions into
    NoOp chains (this toolchain's walrus accepts one wait per instruction).
"""

import numpy as np

import concourse.bass as bass
import concourse.tile as tile
from concourse import mybir
from concourse.bass import ds
from concourse.bass_utils import run_bass_kernel_spmd

# ---------------------------------------------------------------- constants
H = 1024
NH = 8
DH = 128
IN = 512
CD = 256
OUT = 512
FH = 128
B, T = 2048, 32
D = IN + CD  # 768

NCORES = 8
B_LOC = B // NCORES          # 256
NTOK = B_LOC * T             # 8192 tokens per core
TT = 512                     # tokens per tile
NBLK = TT // 128             # 128-token blocks per tile (= 4)

FP32 = mybir.dt.float32
BF16 = mybir.dt.bfloat16
AF = mybir.ActivationFunctionType
ALU = mybir.AluOpType

KO_X = D // 128              # 6 feature chunks of x
KO_H = H // 128              # 8 feature chunks of hidden

# packed per-feature vectors: name -> n_cols (=len/128) in the "vecs" input
VEC_SPECS = [
    ("eb1", 8), ("eg1", 8), ("ebt1", 8),
    ("eb2", 8), ("eg2", 8), ("ebt2", 8),
    ("rb", 8),
    ("lnq_g", 8), ("lnq_b", 8),
    ("cn_g", 8), ("cn_b", 8),
    ("hb1", 8),
    ("fb1", 1), ("fb2", 16),
]
VEC_OFF = {}
_off = 0
for _name, _n in VEC_SPECS:
    VEC_OFF[_name] = _off
    _off += _n
VEC_COLS = _off


# ---------------------------------------------------------------- program
def build_program(ntok=NTOK, tt=TT):
    import concourse.tile_sem_assignment as _tsa
    _tsa.NUM_HWDGE_SEMS = 2
    nt = ntok // tt
    nblk = tt // 128
    nc = bass.Bass()

    # DRAM parameters ------------------------------------------------------
    x_fm = nc.declare_dram_parameter("x_fm", [D, ntok], BF16, isOutput=False)
    vecs_d = nc.declare_dram_parameter("vecs", [128, VEC_COLS], FP32, isOutput=False)
    hb2bc_d = nc.declare_dram_parameter("hb2bc", [128, OUT], FP32, isOutput=False)
    bdt_d = nc.declare_dram_parameter("bdt", [128, NH * 128], BF16, isOutput=False)
    w_d = {}
    for name, k, m in [
        ("ew1", D, H), ("ew2", H, H), ("rw", D, H),
        ("fw1", CD, FH), ("fw2", FH, 2 * H),
        ("wq", H, H), ("wk", H, H), ("wv", H, H), ("wo", H, H),
        ("hw1", H, H), ("hw2", H, OUT),
    ]:
        w_d[name] = nc.declare_dram_parameter(name, [k, m], BF16, isOutput=False)
    out_d = nc.declare_dram_parameter("out_tm", [ntok, OUT], FP32, isOutput=True)

    from contextlib import ExitStack

    with tile.TileContext(nc) as tc, ExitStack() as st:
        singles = st.enter_context(tc.tile_pool(name="singles", bufs=1))
        acts = st.enter_context(tc.tile_pool(name="acts", bufs=1))
        wpool = st.enter_context(tc.tile_pool(name="wpool", bufs=3))
        tmps = st.enter_context(tc.tile_pool(name="tmps", bufs=2))
        stat = st.enter_context(tc.tile_pool(name="stat", bufs=2))
        outp = st.enter_context(tc.tile_pool(name="outp", bufs=2))
        attp = st.enter_context(tc.tile_pool(name="attp", bufs=2))
        psum = st.enter_context(tc.tile_pool(name="psum", bufs=3, space="PSUM"))
        psatt = st.enter_context(tc.tile_pool(name="psatt", bufs=2, space="PSUM"))
        psout = st.enter_context(tc.tile_pool(name="psout", bufs=1, space="PSUM"))

        # resident constants ----------------------------------------------
        vecs = singles.tile([128, VEC_COLS], FP32)
        nc.sync.dma_start(vecs, vecs_d[:, :])
        hb2bc = singles.tile([128, OUT], FP32)
        nc.sync.dma_start(hb2bc, hb2bc_d[:, :])
        bdt = singles.tile([128, NH * 128], BF16)
        nc.sync.dma_start(bdt, bdt_d[:, :])
        ones = singles.tile([128, 128], BF16)
        nc.vector.memset(ones, 1.0)
        eps_sb = singles.tile([128, 1], FP32)
        nc.vector.memset(eps_sb, 1e-5)

        def vec(name, c):
            return vecs[:, VEC_OFF[name] + c : VEC_OFF[name] + c + 1]

        # weight streaming: load a [128, ko_n, m_n] slab of W
        def load_w(name, ko0, ko_n, m0, m_n):
            k_dim = w_d[name].shape[0]
            m_dim = w_d[name].shape[1]
            w3 = w_d[name].rearrange("(ko p) m -> p ko m", p=128)
            t = wpool.tile([128, ko_n, m_n], BF16, tag="w")
            nc.sync.dma_start(t, w3[:, ko0 : ko0 + ko_n, m0 : m0 + m_n])
            return t

        # dense feature-major layer: act_chunks (list of [128, tt] APs) @ W.
        # consumer(mi, ps) receives each output chunk's psum [128, tt].
        # Weights arrive in half-K slabs of 512 output cols for pipelining.
        def fm_layer(name, KO, M, act_of, consumer):
            kh = (KO + 1) // 2  # K-halves: 3+3 for 768, 4+4 for 1024
            for m0 in range(0, M, 512):
                m_n = min(512, M - m0)
                slabs = []
                for k0 in range(0, KO, kh):
                    k_n = min(kh, KO - k0)
                    slabs.append((k0, load_w(name, k0, k_n, m0, m_n)))
                for mi in range(m_n // 128):
                    ps = psum.tile([128, tt], FP32, tag="mm")
                    for k0, slab in slabs:
                        k_n = slab.shape[1]
                        for kk in range(k_n):
                            ko = k0 + kk
                            nc.tensor.matmul(
                                ps,
                                lhsT=slab[:, kk, mi * 128 : (mi + 1) * 128],
                                rhs=act_of(ko),
                                start=(ko == 0),
                                stop=(ko == KO - 1),
                            )
                    consumer(m0 // 128 + mi, ps)

        # LN stats over KO chunks of y -> (rstd, mrs) both [128, tt],
        # broadcast over partitions.  nfeat = KO*128.
        def ln_stats(y_of, KO):
            nfeat = float(KO * 128)
            ps_sum = psum.tile([128, tt], FP32, tag="mm")
            for c in range(KO):
                nc.tensor.matmul(ps_sum, lhsT=ones, rhs=y_of(c),
                                 start=(c == 0), stop=(c == KO - 1))
            ps_ssq = psum.tile([128, tt], FP32, tag="mm")
            for c in range(KO):
                sq = tmps.tile([128, tt], BF16, tag="sq")
                nc.scalar.square(sq, y_of(c))
                nc.tensor.matmul(ps_ssq, lhsT=ones, rhs=sq,
                                 start=(c == 0), stop=(c == KO - 1))
            mean = stat.tile([128, tt], FP32, tag="mean")
            nc.scalar.mul(mean, ps_sum, 1.0 / nfeat)
            var = stat.tile([128, tt], FP32, tag="var")
            nc.scalar.mul(var, ps_ssq, 1.0 / nfeat)
            m2 = stat.tile([128, tt], FP32, tag="m2")
            nc.vector.tensor_mul(m2, mean, mean)
            nc.vector.tensor_sub(var, var, m2)
            # rstd = 1/sqrt(var + eps)
            nc.scalar.activation(var, var, AF.Sqrt, bias=eps_sb, scale=1.0)
            rstd = stat.tile([128, tt], FP32, tag="rstd")
            nc.vector.reciprocal(rstd, var)
            mrs = stat.tile([128, tt], FP32, tag="mrs")
            nc.vector.tensor_mul(mrs, mean, rstd)
            return rstd, mrs

        # normalized chunk, all on DVE (walrus's 2-wait/instruction limit
        # trips on ACT applies that wait on {ACT-seq, DVE, PE} at once)
        def ln_apply_chunk(y_c, rstd, mrs, gname, bname, c, out_c, relu):
            t = tmps.tile([128, tt], FP32, tag="lnt")
            nc.vector.tensor_mul(t, y_c, rstd)
            nc.vector.tensor_sub(t, t, mrs)
            nc.vector.tensor_scalar(out_c, t, vec(gname, c), vec(bname, c),
                                    ALU.mult, ALU.add)
            if relu:
                nc.vector.tensor_scalar_max(out_c, out_c, 0.0)

        # ---------------------------------------------------------- loop
        with tc.For_i(0, ntok, tt) as it:
            x_sb = acts.tile([128, KO_X, tt], BF16, tag="slotA")  # x
            xv = x_fm.rearrange("(kc p) n -> p kc n", p=128)
            nc.sync.dma_start(x_sb, xv[:, :, ds(it, tt)])

            # ---- encoder layer 1: y1 = x@ew1 + eb1 -----------------------
            y_sb = acts.tile([128, KO_H, tt], BF16, tag="slotB")  # y1/y2/s
            fm_layer("ew1", KO_X, H, lambda ko: x_sb[:, ko, :],
                     lambda mi, ps: nc.scalar.activation(
                         y_sb[:, mi, :], ps, AF.Identity, bias=vec("eb1", mi)))
            rstd1, mrs1 = ln_stats(lambda c: y_sb[:, c, :], KO_H)

            # ---- res = x@rw + rb (independent PE work under LN1 chain) ---
            res_sb = acts.tile([128, KO_H, tt], BF16, tag="slotC")  # res/h2
            fm_layer("rw", KO_X, H, lambda ko: x_sb[:, ko, :],
                     lambda mi, ps: nc.scalar.activation(
                         res_sb[:, mi, :], ps, AF.Identity, bias=vec("rb", mi)))

            # ---- FiLM from cond (x chunks 4,5) ---------------------------
            fw1_sb = load_w("fw1", 0, 2, 0, FH)
            psf = psum.tile([128, tt], FP32, tag="mm")
            for kc in range(2):
                nc.tensor.matmul(psf, lhsT=fw1_sb[:, kc, :],
                                 rhs=x_sb[:, 4 + kc, :],
                                 start=(kc == 0), stop=(kc == 1))
            # gelu via tanh approximation (abs err <= 3e-4 on fh; attenuated
            # ~15x through the 0.02-scale fw2 before tanh, so far below the
            # tf32 matmul noise floor). Keeps sim == hw exactly.
            fh_sb = tmps.tile([128, tt], BF16, tag="fh")
            xx = tmps.tile([128, tt], FP32, tag="gelu_x")
            nc.scalar.activation(xx, psf, AF.Identity, bias=vec("fb1", 0))
            x2 = tmps.tile([128, tt], FP32, tag="gelu_t")
            nc.vector.tensor_mul(x2, xx, xx)
            nc.vector.tensor_mul(x2, x2, xx)  # x^3
            nc.vector.scalar_tensor_tensor(x2, x2, 0.044715, xx,
                                           ALU.mult, ALU.add)
            nc.scalar.activation(x2, x2, AF.Tanh, scale=0.7978845608028654)
            nc.vector.tensor_scalar(x2, x2, 0.5, 0.5, ALU.mult, ALU.add)
            nc.vector.tensor_mul(fh_sb, x2, xx)

            g_sb = acts.tile([128, KO_H, tt], BF16, tag="slotD")  # g/k
            b_sb = acts.tile([128, KO_H, tt], BF16, tag="slotE")  # b/v/m
            for half in range(2):
                w2 = load_w("fw2", 0, 1, half * H, H)
                for mi in range(KO_H):
                    ps = psum.tile([128, tt], FP32, tag="mm")
                    nc.tensor.matmul(ps, lhsT=w2[:, 0, mi * 128 : (mi + 1) * 128],
                                     rhs=fh_sb, start=True, stop=True)
                    tn = tmps.tile([128, tt], FP32, tag="lnt")
                    nc.scalar.activation(tn, ps, AF.Tanh,
                                         bias=vec("fb2", half * 8 + mi))
                    if half == 0:
                        nc.vector.tensor_scalar(g_sb[:, mi, :], tn, 0.5, 1.0,
                                                ALU.mult, ALU.add)
                    else:
                        nc.vector.tensor_scalar_mul(b_sb[:, mi, :], tn, 0.5)

            # ---- apply LN1 -> h1 = relu(.) ------------------------------
            h1_sb = acts.tile([128, KO_H, tt], BF16, tag="slotF")  # h1/qin
            for c in range(KO_H):
                ln_apply_chunk(y_sb[:, c, :], rstd1, mrs1, "eg1", "ebt1", c,
                               h1_sb[:, c, :], True)

            # ---- encoder layer 2 + LN2 -> h ------------------------------
            fm_layer("ew2", KO_H, H, lambda ko: h1_sb[:, ko, :],
                     lambda mi, ps: nc.scalar.activation(
                         y_sb[:, mi, :], ps, AF.Identity, bias=vec("eb2", mi)))
            rstd2, mrs2 = ln_stats(lambda c: y_sb[:, c, :], KO_H)
            h_sb = acts.tile([128, KO_H, tt], BF16, tag="slotG")  # h/hr
            for c in range(KO_H):
                ln_apply_chunk(y_sb[:, c, :], rstd2, mrs2, "eg2", "ebt2", c,
                               h_sb[:, c, :], True)

            # ---- qin = LN_q(h)*g + b ------------------------------------
            rstdq, mrsq = ln_stats(lambda c: h_sb[:, c, :], KO_H)
            qin_sb = h1_sb  # h1 dead; reuse the slot handle directly
            for c in range(KO_H):
                t = tmps.tile([128, tt], FP32, tag="lnt")
                nc.vector.tensor_mul(t, h_sb[:, c, :], rstdq)
                nc.vector.tensor_sub(t, t, mrsq)
                u = tmps.tile([128, tt], FP32, tag="lnu")
                nc.vector.tensor_scalar(u, t, vec("lnq_g", c), vec("lnq_b", c),
                                        ALU.mult, ALU.add)
                nc.vector.tensor_mul(u, u, g_sb[:, c, :])
                nc.vector.tensor_add(qin_sb[:, c, :], u, b_sb[:, c, :])

            # ---- k = h@wk (emitted first: covers LNq chain on PE) --------
            k_sb = g_sb  # g dead after qin
            fm_layer("wk", KO_H, H, lambda ko: h_sb[:, ko, :],
                     lambda mi, ps: nc.scalar.copy(k_sb[:, mi, :], ps))

            # ---- q = qin@wq ---------------------------------------------
            q_sb = x_sb  # x dead after res/film
            kq = acts.tile([128, 2, tt], BF16, tag="slotA2")  # q chunks 6,7
            def q_out(mi, ps):
                if mi < KO_X:
                    nc.scalar.copy(q_sb[:, mi, :], ps)
                else:
                    nc.scalar.copy(kq[:, mi - KO_X, :], ps)
            fm_layer("wq", KO_H, H, lambda ko: qin_sb[:, ko, :], q_out)

            def q_chunk(hd):
                return q_sb[:, hd, :] if hd < KO_X else kq[:, hd - KO_X, :]

            # ---- v (token-major): lhsT = h chunk, rhs = wv slab ----------
            # v (token-major) reuses b's slot: block g of 128 tokens stores its
            # [128 tok, 1024 feat] as chunks (g*2, g*2+1) of 512 feature cols.
            v_sb = b_sb  # b dead after qin
            for g in range(nblk):
                for half in range(2):
                    wv_sb = load_w("wv", 0, 4, half * 512, 512)
                    wv_sb2 = load_w("wv", 4, 4, half * 512, 512)
                    ps = psum.tile([128, tt], FP32, tag="mm")
                    for ko in range(KO_H):
                        slab = wv_sb if ko < 4 else wv_sb2
                        nc.tensor.matmul(
                            ps,
                            lhsT=h_sb[:, ko, g * 128 : (g + 1) * 128],
                            rhs=slab[:, ko % 4, :],
                            start=(ko == 0), stop=(ko == KO_H - 1))
                    nc.scalar.copy(v_sb[:, g * 2 + half, :], ps)

            def v_blk(g, hd):
                # v for block g, head hd: [128 tokens, 128 dh]
                ch = g * 2 + hd // 4
                return v_sb[:, ch, (hd % 4) * 128 : (hd % 4 + 1) * 128]

            # ---- hr = h + res (in place into h) --------------------------
            for c in range(KO_H):
                nc.vector.tensor_add(h_sb[:, c, :], h_sb[:, c, :],
                                     res_sb[:, c, :])

            # ---- attention per 128-token block (4 batch items) -----------
            s_sb = y_sb  # y dead after LN2 apply
            for g in range(nblk):
                ps_s = psatt.tile([128, NH * 128], FP32, tag="att")
                for hd in range(NH):
                    for j in range(4):
                        nc.tensor.matmul(
                            ps_s[:, hd * 128 + j * 32 : hd * 128 + (j + 1) * 32],
                            lhsT=k_sb[:, hd, g * 128 : (g + 1) * 128],
                            rhs=q_chunk(hd)[:, g * 128 + j * 32 : g * 128 + (j + 1) * 32],
                            start=True, stop=True)
                exps = attp.tile([128, NH * 128], BF16, tag="exps")
                nc.scalar.activation(exps, ps_s, AF.Exp,
                                     scale=float(1.0 / np.sqrt(DH)))
                ps_d = psatt.tile([128, NH * 128], FP32, tag="att")
                for half in range(2):
                    sl = slice(half * 512, (half + 1) * 512)
                    nc.tensor.matmul(ps_d[:, sl], lhsT=bdt[:, :128],
                                     rhs=exps[:, sl], start=True, stop=True)
                rec = attp.tile([128, NH * 128], FP32, tag="rec", bufs=1)
                nc.vector.reciprocal(rec, ps_d)
                nc.vector.tensor_mul(rec, rec, bdt)
                nc.vector.tensor_mul(exps, exps, rec)
                for hb in range(2):
                    ps_o = psout.tile([128, 512], FP32, tag="opsum")
                    for hh in range(4):
                        hd = hb * 4 + hh
                        nc.tensor.matmul(
                            ps_o[:, hh * 128 : (hh + 1) * 128],
                            lhsT=v_blk(g, hd),
                            rhs=exps[:, hd * 128 : (hd + 1) * 128],
                            start=True, stop=True)
                    for hh in range(4):
                        hd = hb * 4 + hh
                        # s = o (will add wo result later? no: s = h+res+o@wo)
                        nc.scalar.copy(
                            s_sb[:, hd, g * 128 : (g + 1) * 128],
                            ps_o[:, hh * 128 : (hh + 1) * 128])

            # ---- ao = o@wo ; s = hr + ao --------------------------------
            o_sb = s_sb  # naming: s_sb currently holds o (feature-major)
            s2_sb = res_sb  # res dead (folded into hr); reuse for s
            fm_layer("wo", KO_H, H, lambda ko: o_sb[:, ko, :],
                     lambda mi, ps: nc.vector.tensor_add(
                         s2_sb[:, mi, :], ps, h_sb[:, mi, :]))

            # ---- h2 = LN_cn(s) ------------------------------------------
            rstdc, mrsc = ln_stats(lambda c: s2_sb[:, c, :], KO_H)
            h2_sb = h_sb  # hr dead after wo-add
            for c in range(KO_H):
                ln_apply_chunk(s2_sb[:, c, :], rstdc, mrsc, "cn_g", "cn_b", c,
                               h2_sb[:, c, :], False)

            # ---- m = relu(h2@hw1 + hb1) ---------------------------------
            m_sb = v_sb  # v dead after attention
            fm_layer("hw1", KO_H, H, lambda ko: h2_sb[:, ko, :],
                     lambda mi, ps: nc.scalar.activation(
                         m_sb[:, mi, :], ps, AF.Relu, bias=vec("hb1", mi)))

            # ---- out (token-major): lhsT = m chunk, rhs = hw2 -----------
            hw2a = load_w("hw2", 0, 4, 0, OUT)
            hw2b = load_w("hw2", 4, 4, 0, OUT)
            out_sb = outp.tile([128, nblk, OUT], FP32, tag="out", bufs=1)
            for g in range(nblk):
                ps = psum.tile([128, tt], FP32, tag="mm")
                for ko in range(KO_H):
                    slab = hw2a if ko < 4 else hw2b
                    nc.tensor.matmul(
                        ps[:, :OUT],
                        lhsT=m_sb[:, ko, g * 128 : (g + 1) * 128],
                        rhs=slab[:, ko % 4, :],
                        start=(ko == 0), stop=(ko == KO_H - 1))
                nc.vector.tensor_add(out_sb[:, g, :], ps[:, :OUT], hb2bc)
            ov = out_d[ds(it, tt), :].rearrange("(g p) f -> p g f", p=128)
            nc.sync.dma_start(ov, out_sb)

    return nc


# ------------------------------------------------------- walrus wait limit
# The walrus build in this toolchain accepts at most ONE sync-wait command
# per instruction ("Too many sync wait commands" otherwise), while Tile
# emits up to 2 on pipelined instructions and one-per-proc on its tail
# drain. Fix at the BIR-JSON boundary, keeping the bass program (and the
# CoreSim path) untouched:
#   1. drop waits on the instruction's own engine proc for in-order compute
#      instructions (always satisfied: engines complete in PC order);
#   2. peel remaining excess waits onto fresh same-engine NoOps inserted
#      just before the instruction -- the queue blocks on each in turn,
#      which is semantically identical.
_ENGINE_PROCS = ("Activation", "DVE", "PE", "Pool", "SP")
_DMA_OPS = ("DMACopy", "DMATranspose", "TriggeredCopy")


def _rewrite_bir_waits(j):
    n_new = 0
    for fn in j.get("functions", []):
        for bb in fn.get("blocks", []):
            out = []
            for inst in bb.get("instructions", []):
                si = inst.get("sync_info")
                waits = (si or {}).get("on_wait") or []
                if len(waits) > 1:
                    eng = inst.get("engine")
                    opc = inst.get("opcode", "")
                    if (eng in _ENGINE_PROCS and opc not in _DMA_OPS
                            and not opc.startswith("DMA")):
                        own = eng + "_"
                        kept = [w for w in waits
                                if not (w.get("ant_name", "").startswith(own)
                                        and w["ant_name"][len(own):].isdigit())]
                        if kept:
                            waits = kept
                    for w in waits[:-1]:
                        out.append({
                            "debug": inst.get("debug"),
                            "engine": inst["engine"],
                            "ins": [], "outs": [],
                            "name": f"WSPLIT-{n_new}",
                            "opcode": "NoOp",
                            "sync_info": {"on_wait": [w], "on_update": []},
                        })
                        n_new += 1
                    si["on_wait"] = [waits[-1]]
                out.append(inst)
            bb["instructions"] = out
    return j, n_new


def _install_wait_splitter():
    import orjson
    import concourse.bass2jax as b2j
    if getattr(b2j, "_wait_split_installed", False):
        return
    orig = b2j.compile_bir_kernel

    def wrapped(bir_json, *args, **kwargs):
        j = orjson.loads(bir_json)
        j, n_new = _rewrite_bir_waits(j)
        return orig(orjson.dumps(j), *args, **kwargs)

    b2j.compile_bir_kernel = wrapped
    b2j._wait_split_installed = True


# ---------------------------------------------------------------- host side
BF16NP = mybir.dt.np(mybir.dt.bfloat16)


def _pack_shared(inputs):
    f32 = lambda a: np.ascontiguousarray(np.asarray(a, dtype=np.float32))
    shared = {}
    vecs = np.zeros((128, VEC_COLS), dtype=np.float32)
    for name, ncols in VEC_SPECS:
        v = f32(inputs[name]).reshape(ncols, 128)
        vecs[:, VEC_OFF[name] : VEC_OFF[name] + ncols] = v.T
    shared["vecs"] = vecs
    shared["hb2bc"] = np.ascontiguousarray(
        np.broadcast_to(f32(inputs["hb2"])[None, :], (128, OUT)))
    bd = np.kron(np.eye(4, dtype=np.float32), np.ones((32, 32), np.float32))
    shared["bdt"] = np.ascontiguousarray(np.tile(bd, (1, NH))).astype(BF16NP)
    for name in ("ew1", "ew2", "rw", "fw1", "fw2", "wq", "wk", "wv", "wo",
                 "hw1", "hw2"):
        shared[name] = f32(inputs[name]).astype(BF16NP)
    return shared


def make_in_maps(inputs, ncores=NCORES, ntok=NTOK):
    shared = _pack_shared(inputs)
    state = np.asarray(inputs["state"], dtype=np.float32)
    cond = np.asarray(inputs["cond"], dtype=np.float32)
    b_loc = state.shape[0] // ncores
    in_maps = []
    for c in range(ncores):
        sl = slice(c * b_loc, (c + 1) * b_loc)
        x = np.concatenate(
            [state[sl].reshape(-1, IN), cond[sl].reshape(-1, CD)], axis=1)
        np.clip(x, -16.0, 16.0, out=x)
        in_maps.append({"x_fm": np.ascontiguousarray(x.T).astype(BF16NP), **shared})
    return in_maps


_CACHE = {}


def _get_program(ntok=NTOK, tt=TT):
    key = (ntok, tt)
    if key not in _CACHE:
        _CACHE[key] = build_program(ntok, tt)
    return _CACHE[key]


def run(inputs, trace=False):
    """Run on 8 NeuronCores; returns (output [B,T,OUT], BassKernelResults)."""
    _install_wait_splitter()
    nc = _get_program()
    in_maps = make_in_maps(inputs)
    res = run_bass_kernel_spmd(nc, in_maps, list(range(NCORES)), trace=trace)
    outs = [res.results[c]["out_tm"].reshape(B_LOC, T, OUT)
            for c in range(NCORES)]
    return np.concatenate(outs, axis=0), res


def kernel(**inputs) -> np.ndarray:
    out, _ = run(inputs)
    return out



# revision 1
# speedup vs baseline: 1.0134x; 1.0134x over previous
"""
Trainium2 Bass kernel for nn_EventMotionModel (dense transformer block).

Math (per token, B*T=65536 tokens total, hidden H=1024):
    x   = concat(state, cond)            # clip(+-16) is a provable no-op for randn inputs
    h1  = relu(LN(x @ ew1 + eb1))
    h   = relu(LN(h1 @ ew2 + eb2))
    res = x @ rw + rb
    fh  = gelu(cond @ fw1 + fb1); g,b = split(fh @ fw2 + fb2)
    qin = LN_q(h) * (1+0.5*tanh(g)) + 0.5*tanh(b)
    q/k/v = qin@wq, h@wk, h@wv ; per-head (8 heads, dh=128) attention over T=32
    h2  = LN(h + attn_out@wo + res)
    out = relu(h2 @ hw1 + hb1) @ hw2 + hb2

Kernel strategy:
  * Pure batch data parallel over 8 NeuronCores (B=2048 -> 256 per core).
  * Feature-major activations in SBUF: [feature -> partitions (8 chunks of
    128), tokens -> free dim]. Every dense layer is then
        matmul(psum[m,tok], lhsT=W[kchunk, mchunk], rhs=act[kchunk, tok])
    with the weight in its natural [in,out] layout and NO transposes anywhere.
    The host pre-transposes x once; the V projection and the final matmul use
    the activation as the stationary operand, which emits token-major output
    directly (so V is ready for attention and the result DMAs out contiguous).
  * Matmul operands are bf16 (full 78.6 TF/s PE rate; plain fp32 matmul is
    4x slower and its fused 4-byte weight-load path cannot carry the multiple
    sync waits Tile emits). PSUM accumulation and the whole LayerNorm /
    softmax normalization chain stay fp32.
  * LayerNorm mean/var across the partition (feature) dim via an all-ones
    stationary matmul (accumulated over the 8 feature chunks) -> per-token
    sums broadcast across all 128 partitions; x^2 via ACT Square.
  * Attention (T=32 per batch item): scores computed transposed
    [k-tokens, q-tokens] straight from feature-major k (stationary) and q
    (moving). Softmax over the partition dim: exp on ACT (no max subtraction:
    |scores| < ~2 for these weight scales), denominators via a block-diagonal
    ones matmul, then masked reciprocal multiply. o = attn @ v uses
    token-major v as stationary, emitting feature-major o chunks.
  * 16 token tiles of 512 per core via tc.For_i; all weights (16.5MB bf16)
    stream from HBM every tile through a triple-buffered pool, hidden under
    PE work. A BIR-JSON post-pass splits multi-sync-wait instructions into
    NoOp chains (this toolchain's walrus accepts one wait per instruction).
"""

import numpy as np

import concourse.bass as bass
import concourse.tile as tile
from concourse import mybir
from concourse.bass import ds
from concourse.bass_utils import run_bass_kernel_spmd

# ---------------------------------------------------------------- constants
H = 1024
NH = 8
DH = 128
IN = 512
CD = 256
OUT = 512
FH = 128
B, T = 2048, 32
D = IN + CD  # 768

NCORES = 8
B_LOC = B // NCORES          # 256
NTOK = B_LOC * T             # 8192 tokens per core
TT = 512                     # tokens per tile
NBLK = TT // 128             # 128-token blocks per tile (= 4)

FP32 = mybir.dt.float32
BF16 = mybir.dt.bfloat16
AF = mybir.ActivationFunctionType
ALU = mybir.AluOpType

KO_X = D // 128              # 6 feature chunks of x
KO_H = H // 128              # 8 feature chunks of hidden

# packed per-feature vectors: name -> n_cols (=len/128) in the "vecs" input
VEC_SPECS = [
    ("eb1", 8), ("eg1", 8), ("ebt1", 8),
    ("eb2", 8), ("eg2", 8), ("ebt2", 8),
    ("rb", 8),
    ("lnq_g", 8), ("lnq_b", 8),
    ("cn_g", 8), ("cn_b", 8),
    ("hb1", 8),
    ("fb1", 1), ("fb2", 16),
]
VEC_OFF = {}
_off = 0
for _name, _n in VEC_SPECS:
    VEC_OFF[_name] = _off
    _off += _n
VEC_COLS = _off


# ---------------------------------------------------------------- program
def build_program(ntok=NTOK, tt=TT):
    import concourse.tile_sem_assignment as _tsa
    _tsa.NUM_HWDGE_SEMS = 2
    nt = ntok // tt
    nblk = tt // 128
    nc = bass.Bass()

    # DRAM parameters ------------------------------------------------------
    x_fm = nc.declare_dram_parameter("x_fm", [D, ntok], BF16, isOutput=False)
    vecs_d = nc.declare_dram_parameter("vecs", [128, VEC_COLS], FP32, isOutput=False)
    hb2bc_d = nc.declare_dram_parameter("hb2bc", [128, OUT], FP32, isOutput=False)
    bdt_d = nc.declare_dram_parameter("bdt", [128, NH * 128], BF16, isOutput=False)
    w_d = {}
    for name, k, m in [
        ("ew1", D, H), ("ew2", H, H), ("rw", D, H),
        ("fw1", CD, FH), ("fw2", FH, 2 * H),
        ("wq", H, H), ("wk", H, H), ("wv", H, H), ("wo", H, H),
        ("hw1", H, H), ("hw2", H, OUT),
    ]:
        w_d[name] = nc.declare_dram_parameter(name, [k, m], BF16, isOutput=False)
    out_d = nc.declare_dram_parameter("out_tm", [ntok, OUT], FP32, isOutput=True)

    from contextlib import ExitStack

    with tile.TileContext(nc) as tc, ExitStack() as st:
        singles = st.enter_context(tc.tile_pool(name="singles", bufs=1))
        acts = st.enter_context(tc.tile_pool(name="acts", bufs=1))
        wpool = st.enter_context(tc.tile_pool(name="wpool", bufs=3))
        tmps = st.enter_context(tc.tile_pool(name="tmps", bufs=2))
        stat = st.enter_context(tc.tile_pool(name="stat", bufs=2))
        outp = st.enter_context(tc.tile_pool(name="outp", bufs=2))
        attp = st.enter_context(tc.tile_pool(name="attp", bufs=2))
        psum = st.enter_context(tc.tile_pool(name="psum", bufs=3, space="PSUM"))
        psatt = st.enter_context(tc.tile_pool(name="psatt", bufs=2, space="PSUM"))
        psout = st.enter_context(tc.tile_pool(name="psout", bufs=1, space="PSUM"))

        # resident constants ----------------------------------------------
        vecs = singles.tile([128, VEC_COLS], FP32)
        nc.sync.dma_start(vecs, vecs_d[:, :])
        hb2bc = singles.tile([128, OUT], FP32)
        nc.sync.dma_start(hb2bc, hb2bc_d[:, :])
        bdt = singles.tile([128, NH * 128], BF16)
        nc.sync.dma_start(bdt, bdt_d[:, :])
        ones = singles.tile([128, 128], BF16)
        nc.vector.memset(ones, 1.0)
        eps_sb = singles.tile([128, 1], FP32)
        nc.vector.memset(eps_sb, 1e-5)

        def vec(name, c):
            return vecs[:, VEC_OFF[name] + c : VEC_OFF[name] + c + 1]

        # weight streaming: load a [128, ko_n, m_n] slab of W
        def load_w(name, ko0, ko_n, m0, m_n):
            k_dim = w_d[name].shape[0]
            m_dim = w_d[name].shape[1]
            w3 = w_d[name].rearrange("(ko p) m -> p ko m", p=128)
            t = wpool.tile([128, ko_n, m_n], BF16, tag="w")
            nc.sync.dma_start(t, w3[:, ko0 : ko0 + ko_n, m0 : m0 + m_n])
            return t

        # dense feature-major layer: act_chunks (list of [128, tt] APs) @ W.
        # consumer(mi, ps) receives each output chunk's psum [128, tt].
        # Weights arrive in half-K slabs of 512 output cols for pipelining.
        def fm_layer(name, KO, M, act_of, consumer):
            kh = (KO + 1) // 2  # K-halves: 3+3 for 768, 4+4 for 1024
            for m0 in range(0, M, 512):
                m_n = min(512, M - m0)
                slabs = []
                for k0 in range(0, KO, kh):
                    k_n = min(kh, KO - k0)
                    slabs.append((k0, load_w(name, k0, k_n, m0, m_n)))
                for mi in range(m_n // 128):
                    ps = psum.tile([128, tt], FP32, tag="mm")
                    for k0, slab in slabs:
                        k_n = slab.shape[1]
                        for kk in range(k_n):
                            ko = k0 + kk
                            nc.tensor.matmul(
                                ps,
                                lhsT=slab[:, kk, mi * 128 : (mi + 1) * 128],
                                rhs=act_of(ko),
                                start=(ko == 0),
                                stop=(ko == KO - 1),
                            )
                    consumer(m0 // 128 + mi, ps)

        # LN stats over KO chunks of y -> (rstd, mrs) both [128, tt],
        # broadcast over partitions.  nfeat = KO*128.
        def ln_stats(y_of, KO):
            nfeat = float(KO * 128)
            ps_sum = psum.tile([128, tt], FP32, tag="mm")
            for c in range(KO):
                nc.tensor.matmul(ps_sum, lhsT=ones, rhs=y_of(c),
                                 start=(c == 0), stop=(c == KO - 1))
            ps_ssq = psum.tile([128, tt], FP32, tag="mm")
            for c in range(KO):
                sq = tmps.tile([128, tt], BF16, tag="sq")
                nc.scalar.square(sq, y_of(c))
                nc.tensor.matmul(ps_ssq, lhsT=ones, rhs=sq,
                                 start=(c == 0), stop=(c == KO - 1))
            mean = stat.tile([128, tt], FP32, tag="mean")
            nc.scalar.mul(mean, ps_sum, 1.0 / nfeat)
            var = stat.tile([128, tt], FP32, tag="var")
            nc.scalar.mul(var, ps_ssq, 1.0 / nfeat)
            m2 = stat.tile([128, tt], FP32, tag="m2")
            nc.vector.tensor_mul(m2, mean, mean)
            nc.vector.tensor_sub(var, var, m2)
            # rstd = 1/sqrt(var + eps)
            nc.scalar.activation(var, var, AF.Sqrt, bias=eps_sb, scale=1.0)
            rstd = stat.tile([128, tt], FP32, tag="rstd")
            nc.vector.reciprocal(rstd, var)
            mrs = stat.tile([128, tt], FP32, tag="mrs")
            nc.vector.tensor_mul(mrs, mean, rstd)
            return rstd, mrs

        # normalized chunk, all on DVE (walrus's 2-wait/instruction limit
        # trips on ACT applies that wait on {ACT-seq, DVE, PE} at once)
        def ln_apply_chunk(y_c, rstd, mrs, gname, bname, c, out_c, relu):
            t = tmps.tile([128, tt], FP32, tag="lnt")
            nc.vector.tensor_mul(t, y_c, rstd)
            nc.vector.tensor_sub(t, t, mrs)
            nc.vector.tensor_scalar(out_c, t, vec(gname, c), vec(bname, c),
                                    ALU.mult, ALU.add)
            if relu:
                nc.vector.tensor_scalar_max(out_c, out_c, 0.0)

        # ---------------------------------------------------------- loop
        with tc.For_i(0, ntok, tt) as it:
            x_sb = acts.tile([128, KO_X, tt], BF16, tag="slotA")  # x
            xv = x_fm.rearrange("(kc p) n -> p kc n", p=128)
            nc.sync.dma_start(x_sb, xv[:, :, ds(it, tt)])

            # ---- encoder layer 1: y1 = x@ew1 + eb1 -----------------------
            y_sb = acts.tile([128, KO_H, tt], BF16, tag="slotB")  # y1/y2/s
            fm_layer("ew1", KO_X, H, lambda ko: x_sb[:, ko, :],
                     lambda mi, ps: nc.scalar.activation(
                         y_sb[:, mi, :], ps, AF.Identity, bias=vec("eb1", mi)))
            rstd1, mrs1 = ln_stats(lambda c: y_sb[:, c, :], KO_H)

            # ---- res = x@rw + rb (independent PE work under LN1 chain) ---
            res_sb = acts.tile([128, KO_H, tt], BF16, tag="slotC")  # res/h2
            fm_layer("rw", KO_X, H, lambda ko: x_sb[:, ko, :],
                     lambda mi, ps: nc.scalar.activation(
                         res_sb[:, mi, :], ps, AF.Identity, bias=vec("rb", mi)))

            # ---- FiLM from cond (x chunks 4,5) ---------------------------
            fw1_sb = load_w("fw1", 0, 2, 0, FH)
            psf = psum.tile([128, tt], FP32, tag="mm")
            for kc in range(2):
                nc.tensor.matmul(psf, lhsT=fw1_sb[:, kc, :],
                                 rhs=x_sb[:, 4 + kc, :],
                                 start=(kc == 0), stop=(kc == 1))
            # gelu via tanh approximation (abs err <= 3e-4 on fh; attenuated
            # ~15x through the 0.02-scale fw2 before tanh, so far below the
            # tf32 matmul noise floor). Keeps sim == hw exactly.
            fh_sb = tmps.tile([128, tt], BF16, tag="fh")
            xx = tmps.tile([128, tt], FP32, tag="gelu_x")
            nc.scalar.activation(xx, psf, AF.Identity, bias=vec("fb1", 0))
            x2 = tmps.tile([128, tt], FP32, tag="gelu_t")
            nc.vector.tensor_mul(x2, xx, xx)
            nc.vector.tensor_mul(x2, x2, xx)  # x^3
            nc.vector.scalar_tensor_tensor(x2, x2, 0.044715, xx,
                                           ALU.mult, ALU.add)
            nc.scalar.activation(x2, x2, AF.Tanh, scale=0.7978845608028654)
            nc.vector.tensor_scalar(x2, x2, 0.5, 0.5, ALU.mult, ALU.add)
            nc.vector.tensor_mul(fh_sb, x2, xx)

            g_sb = acts.tile([128, KO_H, tt], BF16, tag="slotD")  # g/k
            b_sb = acts.tile([128, KO_H, tt], BF16, tag="slotE")  # b/v/m
            for half in range(2):
                w2 = load_w("fw2", 0, 1, half * H, H)
                for mi in range(KO_H):
                    ps = psum.tile([128, tt], FP32, tag="mm")
                    nc.tensor.matmul(ps, lhsT=w2[:, 0, mi * 128 : (mi + 1) * 128],
                                     rhs=fh_sb, start=True, stop=True)
                    tn = tmps.tile([128, tt], FP32, tag="lnt")
                    nc.scalar.activation(tn, ps, AF.Tanh,
                                         bias=vec("fb2", half * 8 + mi))
                    if half == 0:
                        nc.vector.tensor_scalar(g_sb[:, mi, :], tn, 0.5, 1.0,
                                                ALU.mult, ALU.add)
                    else:
                        nc.vector.tensor_scalar_mul(b_sb[:, mi, :], tn, 0.5)

            # ---- apply LN1 -> h1 = relu(.) ------------------------------
            h1_sb = acts.tile([128, KO_H, tt], BF16, tag="slotF")  # h1/qin
            for c in range(KO_H):
                ln_apply_chunk(y_sb[:, c, :], rstd1, mrs1, "eg1", "ebt1", c,
                               h1_sb[:, c, :], True)

            # ---- encoder layer 2 + LN2 -> h ------------------------------
            fm_layer("ew2", KO_H, H, lambda ko: h1_sb[:, ko, :],
                     lambda mi, ps: nc.scalar.activation(
                         y_sb[:, mi, :], ps, AF.Identity, bias=vec("eb2", mi)))
            rstd2, mrs2 = ln_stats(lambda c: y_sb[:, c, :], KO_H)
            h_sb = acts.tile([128, KO_H, tt], BF16, tag="slotG")  # h/hr
            for c in range(KO_H):
                ln_apply_chunk(y_sb[:, c, :], rstd2, mrs2, "eg2", "ebt2", c,
                               h_sb[:, c, :], True)

            # ---- qin = LN_q(h)*g + b ------------------------------------
            rstdq, mrsq = ln_stats(lambda c: h_sb[:, c, :], KO_H)
            qin_sb = h1_sb  # h1 dead; reuse the slot handle directly
            for c in range(KO_H):
                t = tmps.tile([128, tt], FP32, tag="lnt")
                nc.vector.tensor_mul(t, h_sb[:, c, :], rstdq)
                nc.vector.tensor_sub(t, t, mrsq)
                u = tmps.tile([128, tt], FP32, tag="lnu")
                nc.vector.tensor_scalar(u, t, vec("lnq_g", c), vec("lnq_b", c),
                                        ALU.mult, ALU.add)
                nc.vector.tensor_mul(u, u, g_sb[:, c, :])
                nc.vector.tensor_add(qin_sb[:, c, :], u, b_sb[:, c, :])

            # ---- k = h@wk (emitted first: covers LNq chain on PE) --------
            k_sb = g_sb  # g dead after qin
            fm_layer("wk", KO_H, H, lambda ko: h_sb[:, ko, :],
                     lambda mi, ps: nc.scalar.copy(k_sb[:, mi, :], ps))

            # ---- q = qin@wq ---------------------------------------------
            q_sb = x_sb  # x dead after res/film
            kq = acts.tile([128, 2, tt], BF16, tag="slotA2")  # q chunks 6,7
            def q_out(mi, ps):
                if mi < KO_X:
                    nc.scalar.copy(q_sb[:, mi, :], ps)
                else:
                    nc.scalar.copy(kq[:, mi - KO_X, :], ps)
            fm_layer("wq", KO_H, H, lambda ko: qin_sb[:, ko, :], q_out)

            def q_chunk(hd):
                return q_sb[:, hd, :] if hd < KO_X else kq[:, hd - KO_X, :]

            # ---- v (token-major): lhsT = h chunk, rhs = wv slab ----------
            # v (token-major) reuses b's slot: block g of 128 tokens stores its
            # [128 tok, 1024 feat] as chunks (g*2, g*2+1) of 512 feature cols.
            v_sb = b_sb  # b dead after qin
            for g in range(nblk):
                for half in range(2):
                    wv_sb = load_w("wv", 0, 4, half * 512, 512)
                    wv_sb2 = load_w("wv", 4, 4, half * 512, 512)
                    ps = psum.tile([128, tt], FP32, tag="mm")
                    for ko in range(KO_H):
                        slab = wv_sb if ko < 4 else wv_sb2
                        nc.tensor.matmul(
                            ps,
                            lhsT=h_sb[:, ko, g * 128 : (g + 1) * 128],
                            rhs=slab[:, ko % 4, :],
                            start=(ko == 0), stop=(ko == KO_H - 1))
                    nc.scalar.copy(v_sb[:, g * 2 + half, :], ps)

            def v_blk(g, hd):
                # v for block g, head hd: [128 tokens, 128 dh]
                ch = g * 2 + hd // 4
                return v_sb[:, ch, (hd % 4) * 128 : (hd % 4 + 1) * 128]

            # ---- hr = h + res (in place into h) --------------------------
            for c in range(KO_H):
                nc.vector.tensor_add(h_sb[:, c, :], h_sb[:, c, :],
                                     res_sb[:, c, :])

            # ---- attention per 128-token block (4 batch items) -----------
            s_sb = y_sb  # y dead after LN2 apply
            for g in range(nblk):
                ps_s = psatt.tile([128, NH * 128], FP32, tag="att")
                for hd in range(NH):
                    for j in range(4):
                        nc.tensor.matmul(
                            ps_s[:, hd * 128 + j * 32 : hd * 128 + (j + 1) * 32],
                            lhsT=k_sb[:, hd, g * 128 : (g + 1) * 128],
                            rhs=q_chunk(hd)[:, g * 128 + j * 32 : g * 128 + (j + 1) * 32],
                            start=True, stop=True)
                exps = attp.tile([128, NH * 128], BF16, tag="exps")
                nc.scalar.activation(exps, ps_s, AF.Exp,
                                     scale=float(1.0 / np.sqrt(DH)))
                ps_d = psatt.tile([128, NH * 128], FP32, tag="att")
                for half in range(2):
                    sl = slice(half * 512, (half + 1) * 512)
                    nc.tensor.matmul(ps_d[:, sl], lhsT=bdt[:, :128],
                                     rhs=exps[:, sl], start=True, stop=True)
                rec = attp.tile([128, NH * 128], FP32, tag="rec", bufs=1)
                nc.vector.reciprocal(rec, ps_d)
                nc.vector.tensor_mul(rec, rec, bdt)
                nc.vector.tensor_mul(exps, exps, rec)
                for hb in range(2):
                    ps_o = psout.tile([128, 512], FP32, tag="opsum")
                    for hh in range(4):
                        hd = hb * 4 + hh
                        nc.tensor.matmul(
                            ps_o[:, hh * 128 : (hh + 1) * 128],
                            lhsT=v_blk(g, hd),
                            rhs=exps[:, hd * 128 : (hd + 1) * 128],
                            start=True, stop=True)
                    for hh in range(4):
                        hd = hb * 4 + hh
                        # s = o (will add wo result later? no: s = h+res+o@wo)
                        nc.scalar.copy(
                            s_sb[:, hd, g * 128 : (g + 1) * 128],
                            ps_o[:, hh * 128 : (hh + 1) * 128])

            # ---- ao = o@wo ; s = hr + ao --------------------------------
            o_sb = s_sb  # naming: s_sb currently holds o (feature-major)
            s2_sb = res_sb  # res dead (folded into hr); reuse for s
            fm_layer("wo", KO_H, H, lambda ko: o_sb[:, ko, :],
                     lambda mi, ps: nc.vector.tensor_add(
                         s2_sb[:, mi, :], ps, h_sb[:, mi, :]))

            # ---- h2 = LN_cn(s) ------------------------------------------
            rstdc, mrsc = ln_stats(lambda c: s2_sb[:, c, :], KO_H)
            h2_sb = h_sb  # hr dead after wo-add
            for c in range(KO_H):
                ln_apply_chunk(s2_sb[:, c, :], rstdc, mrsc, "cn_g", "cn_b", c,
                               h2_sb[:, c, :], False)

            # ---- m = relu(h2@hw1 + hb1) ---------------------------------
            m_sb = v_sb  # v dead after attention
            fm_layer("hw1", KO_H, H, lambda ko: h2_sb[:, ko, :],
                     lambda mi, ps: nc.scalar.activation(
                         m_sb[:, mi, :], ps, AF.Relu, bias=vec("hb1", mi)))

            # ---- out (token-major): lhsT = m chunk, rhs = hw2 -----------
            hw2a = load_w("hw2", 0, 4, 0, OUT)
            hw2b = load_w("hw2", 4, 4, 0, OUT)
            out_sb = outp.tile([128, nblk, OUT], FP32, tag="out", bufs=1)
            for g in range(nblk):
                ps = psum.tile([128, tt], FP32, tag="mm")
                for ko in range(KO_H):
                    slab = hw2a if ko < 4 else hw2b
                    nc.tensor.matmul(
                        ps[:, :OUT],
                        lhsT=m_sb[:, ko, g * 128 : (g + 1) * 128],
                        rhs=slab[:, ko % 4, :],
                        start=(ko == 0), stop=(ko == KO_H - 1))
                nc.vector.tensor_add(out_sb[:, g, :], ps[:, :OUT], hb2bc)
            ov = out_d[ds(it, tt), :].rearrange("(g p) f -> p g f", p=128)
            nc.sync.dma_start(ov, out_sb)

    return nc


# ------------------------------------------------------- walrus wait limit
# The walrus build in this toolchain accepts at most ONE sync-wait command
# per instruction ("Too many sync wait commands" otherwise), while Tile
# emits up to 2 on pipelined instructions and one-per-proc on its tail
# drain. Fix at the BIR-JSON boundary, keeping the bass program (and the
# CoreSim path) untouched:
#   1. drop waits on the instruction's own engine proc for in-order compute
#      instructions (always satisfied: engines complete in PC order);
#   2. peel remaining excess waits onto fresh same-engine NoOps inserted
#      just before the instruction -- the queue blocks on each in turn,
#      which is semantically identical.
_ENGINE_PROCS = ("Activation", "DVE", "PE", "Pool", "SP")
_DMA_OPS = ("DMACopy", "DMATranspose", "TriggeredCopy")


def _rewrite_bir_waits(j):
    n_new = 0
    for fn in j.get("functions", []):
        for bb in fn.get("blocks", []):
            out = []
            for inst in bb.get("instructions", []):
                si = inst.get("sync_info")
                waits = (si or {}).get("on_wait") or []
                if len(waits) > 1:
                    eng = inst.get("engine")
                    opc = inst.get("opcode", "")
                    if (eng in _ENGINE_PROCS and opc not in _DMA_OPS
                            and not opc.startswith("DMA")):
                        own = eng + "_"
                        kept = [w for w in waits
                                if not (w.get("ant_name", "").startswith(own)
                                        and w["ant_name"][len(own):].isdigit())]
                        if kept:
                            waits = kept
                    for w in waits[:-1]:
                        out.append({
                            "debug": inst.get("debug"),
                            "engine": inst["engine"],
                            "ins": [], "outs": [],
                            "name": f"WSPLIT-{n_new}",
                            "opcode": "NoOp",
                            "sync_info": {"on_wait": [w], "on_update": []},
                        })
                        n_new += 1
                    si["on_wait"] = [waits[-1]]
                out.append(inst)
            bb["instructions"] = out
    return j, n_new


def _install_wait_splitter():
    import orjson
    import concourse.bass2jax as b2j
    if getattr(b2j, "_wait_split_installed", False):
        return
    orig = b2j.compile_bir_kernel

    def wrapped(bir_json, *args, **kwargs):
        j = orjson.loads(bir_json)
        j, n_new = _rewrite_bir_waits(j)
        return orig(orjson.dumps(j), *args, **kwargs)

    b2j.compile_bir_kernel = wrapped
    b2j._wait_split_installed = True


# ---------------------------------------------------------------- host side
BF16NP = mybir.dt.np(mybir.dt.bfloat16)


def _pack_shared(inputs):
    f32 = lambda a: np.ascontiguousarray(np.asarray(a, dtype=np.float32))
    shared = {}
    vecs = np.zeros((128, VEC_COLS), dtype=np.float32)
    for name, ncols in VEC_SPECS:
        v = f32(inputs[name]).reshape(ncols, 128)
        vecs[:, VEC_OFF[name] : VEC_OFF[name] + ncols] = v.T
    shared["vecs"] = vecs
    shared["hb2bc"] = np.ascontiguousarray(
        np.broadcast_to(f32(inputs["hb2"])[None, :], (128, OUT)))
    bd = np.kron(np.eye(4, dtype=np.float32), np.ones((32, 32), np.float32))
    shared["bdt"] = np.ascontiguousarray(np.tile(bd, (1, NH))).astype(BF16NP)
    for name in ("ew1", "ew2", "rw", "fw1", "fw2", "wq", "wk", "wv", "wo",
                 "hw1", "hw2"):
        shared[name] = f32(inputs[name]).astype(BF16NP)
    return shared


def make_in_maps(inputs, ncores=NCORES, ntok=NTOK):
    shared = _pack_shared(inputs)
    state = np.asarray(inputs["state"], dtype=np.float32)
    cond = np.asarray(inputs["cond"], dtype=np.float32)
    b_loc = state.shape[0] // ncores
    in_maps = []
    for c in range(ncores):
        sl = slice(c * b_loc, (c + 1) * b_loc)
        x = np.concatenate(
            [state[sl].reshape(-1, IN), cond[sl].reshape(-1, CD)], axis=1)
        np.clip(x, -16.0, 16.0, out=x)
        in_maps.append({"x_fm": np.ascontiguousarray(x.T).astype(BF16NP), **shared})
    return in_maps


_CACHE = {}


def _get_program(ntok=NTOK, tt=TT):
    key = (ntok, tt)
    if key not in _CACHE:
        _CACHE[key] = build_program(ntok, tt)
    return _CACHE[key]


def run(inputs, trace=False):
    """Run on 8 NeuronCores; returns (output [B,T,OUT], BassKernelResults)."""
    _install_wait_splitter()
    nc = _get_program()
    in_maps = make_in_maps(inputs)
    res = run_bass_kernel_spmd(nc, in_maps, list(range(NCORES)), trace=trace)
    outs = [res.results[c]["out_tm"].reshape(B_LOC, T, OUT)
            for c in range(NCORES)]
    return np.concatenate(outs, axis=0), res


def kernel(**inputs) -> np.ndarray:
    out, _ = run(inputs)
    return out

